# revision 1
# baseline (speedup 1.0000x reference)
# Tropical (max/min-plus) pseudo-matmul kernel for Trainium2, SPMD over 8 cores.
#
#   out[b, u] = max_f(x[b,f] + w[f,u])   for u < 128
#   out[b, u] = min_f(x[b,f] + w[f,u])   for u >= 128
#
# Strategy: map the tropical matmul onto the PE array via the log-sum-exp
# limit.  With per-row/per-col normalizers mx[b], mw[u]:
#
#   max_f(x+w) ~= mx + mw + (1/T) * ( ln( sum_f e^{T(x-mx)+A} * e^{T(w-mw)+A} ) - 2A )
#
# i.e. a plain matmul of exponential factors (bf16) accumulated in fp32.
# T is capped by bf16 factor underflow on the winning term; factors carry a
# +A=+40 exponent shift each so products span e^{+80}..e^{-87}.  The f
# dimension is split into NB sum-blocks that are max-combined in log space
# (exact), removing cross-block competitor mass from the soft-max bias.  The
# min half runs the same pipeline on negated data.  ln() is evaluated by
# splitting S = m * 2^e with integer ops (the ACT Ln table only covers
# 2^[-64,64]) so only the mantissa in [1,2) hits the table.
# Batch is sharded 8 x 256 rows; w is replicated.
import numpy as np
from contextlib import ExitStack

import concourse.bass as bass
import concourse.bacc as bacc
import concourse.tile as tile
from concourse import mybir, bass_isa, library_config
from concourse.bass_utils import run_bass_kernel_spmd
from concourse.masks import make_identity

FP32 = mybir.dt.float32
BF16 = mybir.dt.bfloat16
I32 = mybir.dt.int32
AF = mybir.ActivationFunctionType
ALU = mybir.AluOpType
X_AX = mybir.AxisListType.X

T = 23.25       # LSE sharpness; limited by bf16 factor underflow on real data
ALPHA = 40.0    # per-factor exponent shift
LN2 = float(np.log(2.0))
NB = 2          # number of f sum-blocks (each 2 K-tiles), max-combined in log
N_CORES = 8
BPC = 256       # batch rows per core
F = 512
U = 256
KT = 4          # K tiles of 128


def _patch_act_tables():
    """Make natural_log_exp_and_others the only table set providing Exp/Ln
    so the Bacc table-load pass emits a single ACT_TABLE_LOAD."""
    if getattr(bacc, "_act_tables_patched", False):
        return
    orig = bacc.get_activation_tables

    def patched(arch):
        t = dict(orig(arch))
        for name in list(t.keys()):
            if name != "natural_log_exp_and_others":
                t[name] = set(t[name]) - {AF.Exp, AF.Ln}
        return t

    bacc.get_activation_tables = patched
    bacc._act_tables_patched = True


def _build_module() -> bass.Bass:
    _patch_act_tables()
    nc = bacc.Bacc(None, target_bir_lowering=False)
    x_in = nc.declare_dram_parameter("x", [BPC, F], FP32, isOutput=False)
    w_in = nc.declare_dram_parameter("w", [F, U], FP32, isOutput=False)
    out_ext = nc.declare_dram_parameter("out", [BPC, U], FP32, isOutput=True)

    with tile.TileContext(nc) as tc, ExitStack() as ctx:
        sb = ctx.enter_context(tc.tile_pool(name="sb", bufs=1))
        pst_pool = ctx.enter_context(tc.tile_pool(name="pst", bufs=4, space="PSUM"))
        psS_pool = ctx.enter_context(tc.tile_pool(name="psS", bufs=2, space="PSUM"))

        # ---- loads (two HWDGE rings: x on SP, w on ACT) ----
        wt = sb.tile([128, KT, U], FP32, tag="wt")      # wt[p, k, :] = w[k*128+p, :]
        nc.gpsimd.dma_start(out=wt, in_=w_in.rearrange("(k p) u -> p k u", p=128))
        xt = sb.tile([128, 2, F], FP32, tag="xt")       # xt[p, m, :] = x[m*128+p, :]
        xv = x_in.rearrange("(m p) f -> p m f", p=128)
        nc.sync.dma_start(out=xt[:, 0, :], in_=xv[:, 0, :])
        nc.scalar.dma_start(out=xt[:, 1, :], in_=xv[:, 1, :])

        ident = sb.tile([128, 128], BF16, tag="ident")
        make_identity(nc, ident)
        lnb_col = sb.tile([128, 1], FP32, tag="lnb_col")
        nc.vector.memset(lnb_col, 0.0)

        # ---- w chain (critical path: w -> wts -> tree -> allred -> dif -> ew)
        # wts[:, k, 0:128] = +T*w (max half); wts[:, k, 128:256] = -T*w (min)
        wts = sb.tile([128, KT, U], FP32, tag="wts")
        nc.vector.tensor_scalar(out=wts[:, :, 0:128], in0=wt[:, :, 0:128],
                                scalar1=T, scalar2=None, op0=ALU.mult)
        nc.vector.tensor_scalar(out=wts[:, :, 128:U], in0=wt[:, :, 128:U],
                                scalar1=-T, scalar2=None, op0=ALU.mult)
        # max over the 4 K-tiles, then partition max:
        # wred = [T*mw | -T*mnw] broadcast to all 128 partitions.
        t01 = sb.tile([128, 2, U], FP32, tag="t01")
        comb = sb.tile([128, U], FP32, tag="comb")
        nc.vector.tensor_max(out=t01, in0=wts[:, 0:2, :], in1=wts[:, 2:4, :])
        nc.vector.tensor_max(out=comb, in0=t01[:, 0, :], in1=t01[:, 1, :])

        # ---- x row stats + exp biases ----
        mx = sb.tile([128, 2], FP32, tag="mx")
        mn = sb.tile([128, 2], FP32, tag="mn")
        biasP = sb.tile([128, 2], FP32, tag="biasP")
        biasN = sb.tile([128, 2], FP32, tag="biasN")
        exP = sb.tile([128, 2, F], BF16, tag="exP")
        exN = sb.tile([128, 2, F], BF16, tag="exN")

        def x_stats(m):
            nc.vector.tensor_reduce(out=mx[:, m:m + 1], in_=xt[:, m, :],
                                    axis=X_AX, op=ALU.max)
            nc.vector.tensor_reduce(out=mn[:, m:m + 1], in_=xt[:, m, :],
                                    axis=X_AX, op=ALU.min)
            nc.vector.tensor_scalar(out=biasP[:, m:m + 1], in0=mx[:, m:m + 1],
                                    scalar1=-T, scalar2=ALPHA,
                                    op0=ALU.mult, op1=ALU.add)
            nc.vector.tensor_scalar(out=biasN[:, m:m + 1], in0=mn[:, m:m + 1],
                                    scalar1=T, scalar2=ALPHA,
                                    op0=ALU.mult, op1=ALU.add)

        def x_exps(m):
            nc.scalar.activation(out=exP[:, m, :], in_=xt[:, m, :], func=AF.Exp,
                                 bias=biasP[:, m:m + 1], scale=T)
            nc.scalar.activation(out=exN[:, m, :], in_=xt[:, m, :], func=AF.Exp,
                                 bias=biasN[:, m:m + 1], scale=-T)

        wred = sb.tile([128, U], FP32, tag="wred")
        nc.gpsimd.partition_all_reduce(out_ap=wred, in_ap=comb, channels=128,
                                       reduce_op=bass_isa.ReduceOp.max)

        x_stats(0)
        x_exps(0)

        # w factors: ew[:, k, u] = exp(wts - wred + ALPHA), per K-tile pair
        ew = sb.tile([128, KT, U], BF16, tag="ew")
        dif = sb.tile([128, KT, U], FP32, tag="dif")
        alpha_col = sb.tile([128, 1], FP32, tag="alpha_col")
        nc.vector.memset(alpha_col, ALPHA)
        for kp in range(2):
            sl = slice(2 * kp, 2 * kp + 2)
            nc.vector.tensor_sub(out=dif[:, sl, :], in0=wts[:, sl, :],
                                 in1=wred.rearrange("p (o u) -> p o u", o=1)
                                         .to_broadcast((128, 2, U)))
            nc.scalar.activation(out=ew[:, sl, :], in_=dif[:, sl, :],
                                 func=AF.Exp, bias=alpha_col, scale=1.0)

        x_stats(1)
        x_exps(1)

        # epilogue-adjusted stats: mxadj = mx - 2A/T ; mnadj = mn + 2A/T
        mxadj = sb.tile([128, 2], FP32, tag="mxadj")
        mnadj = sb.tile([128, 2], FP32, tag="mnadj")
        nc.vector.tensor_scalar(out=mxadj, in0=mx,
                                scalar1=(-2.0 * ALPHA - 127.0 * LN2) / T,
                                scalar2=None, op0=ALU.add)
        nc.vector.tensor_scalar(out=mnadj, in0=mn,
                                scalar1=(2.0 * ALPHA + 127.0 * LN2) / T,
                                scalar2=None, op0=ALU.add)

        # ---- transpose x factors to (f, b); 4 per PSUM bank, one copy each
        exT = {}
        for m in range(2):
            for v, ex in enumerate((exP, exN)):
                pstb = pst_pool.tile([128, KT, 128], BF16, tag="pstb")
                for k in range(KT):
                    nc.tensor.transpose(pstb[:, k, :],
                                        ex[:, m, k * 128:(k + 1) * 128], ident)
                dst = sb.tile([128, KT, 128], BF16, tag=f"exT{v}{m}",
                              name=f"exT{v}{m}")
                if v == 0:
                    nc.vector.tensor_copy(out=dst, in_=pstb)
                else:
                    nc.scalar.copy(out=dst, in_=pstb)
                exT[(v, m)] = dst

        # ---- blocked matmuls + fused log-space epilogue, per m ----
        res = [sb.tile([128, U], FP32, tag=f"res{m}", name=f"res{m}")
               for m in range(2)]
        for m in range(2):
            S = psS_pool.tile([128, 2, NB, 128], FP32, tag="S")  # [v, blk, u]
            for v in range(2):
                for k in range(KT):
                    nc.tensor.matmul(
                        out=S[:, v, k // 2, :],
                        lhsT=exT[(v, m)][:, k, :],
                        rhs=ew[:, k, v * 128:(v + 1) * 128],
                        start=(k % 2 == 0), stop=(k % 2 == 1))
            # Sred[v, u] = max over blocks (exact in log space)
            SredM = sb.tile([128, 2, 128], FP32, tag="SredM")
            nc.vector.tensor_reduce(out=SredM,
                                    in_=S.rearrange("p v b u -> p v u b"),
                                    axis=X_AX, op=ALU.max)
            flat = SredM.rearrange("p v u -> p (v u)")
            bits = flat.bitcast(I32)
            # S = mant * 2^(e-127):  ef = (e-127)*ln2 ; mant in [1, 2)
            # ef = e * ln2 (the -127*ln2 constant is folded into mxadj/mnadj)
            ef = sb.tile([128, U], FP32, tag="ef")
            nc.vector.tensor_scalar(out=ef.bitcast(I32), in0=bits,
                                    scalar1=23, scalar2=None,
                                    op0=ALU.arith_shift_right)
            nc.vector.tensor_scalar(out=ef, in0=ef.bitcast(I32),
                                    scalar1=LN2, scalar2=None, op0=ALU.mult)
            mant = sb.tile([128, U], FP32, tag="mant")
            nc.vector.tensor_scalar(out=mant.bitcast(I32), in0=bits,
                                    scalar1=0x007FFFFF, scalar2=0x3F800000,
                                    op0=ALU.bitwise_and, op1=ALU.bitwise_or)
            lnm = sb.tile([128, U], FP32, tag="lnm")
            nc.scalar.activation(out=lnm, in_=mant, func=AF.Ln,
                                 bias=lnb_col, scale=1.0)
            # res = sgn * (lnm + ef + wred) + statadj ; g1 overlaps the Ln
            g1 = sb.tile([128, U], FP32, tag="g1")
            nc.vector.scalar_tensor_tensor(out=g1, in0=ef, scalar=0.0, in1=wred,
                                           op0=ALU.add, op1=ALU.add)
            t3 = sb.tile([128, U], FP32, tag="t3")
            nc.vector.scalar_tensor_tensor(out=t3, in0=lnm, scalar=0.0, in1=g1,
                                           op0=ALU.add, op1=ALU.add)
            for v in range(2):
                sgn = (1.0 / T) if v == 0 else (-1.0 / T)
                stat = mxadj if v == 0 else mnadj
                nc.vector.tensor_scalar(
                    out=res[m][:, v * 128:(v + 1) * 128],
                    in0=t3[:, v * 128:(v + 1) * 128],
                    scalar1=sgn, scalar2=stat[:, m:m + 1],
                    op0=ALU.mult, op1=ALU.add)
                # each half ships as soon as its final op lands
                nc.sync.dma_start(
                    out=out_ext[m * 128:(m + 1) * 128, v * 128:(v + 1) * 128],
                    in_=res[m][:, v * 128:(v + 1) * 128])

    nc.finalize()
    return nc


_NC = None


def _get_module() -> bass.Bass:
    global _NC
    if _NC is None:
        _NC = _build_module()
    return _NC


def kernel(x: np.ndarray, w: np.ndarray, _trace: bool = False, **_unused):
    assert x.shape == (2048, 512) and w.shape == (512, 256)
    x = np.ascontiguousarray(x, dtype=np.float32)
    w = np.ascontiguousarray(w, dtype=np.float32)
    nc = _get_module()
    in_maps = [
        {"x": x[i * BPC:(i + 1) * BPC], "w": w} for i in range(N_CORES)
    ]
    r = run_bass_kernel_spmd(nc, in_maps, list(range(N_CORES)), trace=_trace)
    out = np.concatenate([r.results[i]["out"] for i in range(N_CORES)], axis=0)
    if _trace:
        kernel.last_exec_time_ns = r.exec_time_ns
        kernel.last_results = r
    return out



# revision 7
# speedup vs baseline: 1.0987x; 1.0987x over previous
# Tropical (max/min-plus) pseudo-matmul kernel for Trainium2, SPMD over 8 cores.
#
#   out[b, u] = max_f(x[b,f] + w[f,u])   for u < 128
#   out[b, u] = min_f(x[b,f] + w[f,u])   for u >= 128
#
# Log-sum-exp on the PE array: with per-row/per-col normalizers mx[b], mw[u],
#   max_f(x+w) ~= mx + mw + (1/T)( ln( sum_f e^{T(x-mx)+A} e^{T(w-mw)+A} ) - 2A )
# i.e. a plain bf16 matmul of exponential factors accumulated in fp32.  The min
# half runs the same pipeline on negated data.  ln() is evaluated by splitting
# S = m * 2^e with integer ops so only the mantissa in [1,2) hits the Ln table.
#
# v2 layout choices (vs the earlier 32.8us version):
#  - inputs are cast to bf16 on the host: halves DMA bytes, doubles DVE rates.
#  - per-u normalizer via PE (transpose -> free-axis reduce -> ones-matmul
#    broadcast) instead of gpsimd partition_all_reduce (avoids ~6us IRAM load).
#  - x-side: subtract the row stat first (fp16), PE-transpose the *pre-exp*
#    values, then exp straight out of PSUM -> kills the PSUM->SBUF copies.
#  - single PSUM accumulation group over all 4 K-tiles (no block-max reduce).
#  - PE warm-up matmuls during the DMA window so real MMs run at 2.4 GHz.
# Batch is sharded 8 x 256 rows; w is replicated.
import numpy as np
import ml_dtypes
from contextlib import ExitStack

import concourse.bass as bass
import concourse.bacc as bacc
import concourse.tile as tile
from concourse import mybir
from concourse.bass_utils import run_bass_kernel_spmd
from concourse.masks import make_identity

FP32 = mybir.dt.float32
BF16 = mybir.dt.bfloat16
FP16 = mybir.dt.float16
I32 = mybir.dt.int32
AF = mybir.ActivationFunctionType
ALU = mybir.AluOpType
X_AX = mybir.AxisListType.X

T = 23.25       # LSE sharpness; limited by bf16 factor underflow on real data
ALPHA = 40.0    # per-factor exponent shift
LN2 = float(np.log(2.0))
N_CORES = 8
BPC = 256       # batch rows per core
F = 512
U = 256
KT = 4          # K tiles of 128
NWARM = 24      # PE warm-up matmuls (HAM un-throttle needs ~3.4us of activity)


def _patch_act_tables():
    """Make natural_log_exp_and_others the only table set providing Exp/Ln
    so the Bacc table-load pass emits a single ACT_TABLE_LOAD."""
    if getattr(bacc, "_act_tables_patched", False):
        return
    orig = bacc.get_activation_tables

    def patched(arch):
        t = dict(orig(arch))
        for name in list(t.keys()):
            if name != "natural_log_exp_and_others":
                t[name] = set(t[name]) - {AF.Exp, AF.Ln}
        return t

    bacc.get_activation_tables = patched
    bacc._act_tables_patched = True


def _build_module() -> bass.Bass:
    _patch_act_tables()
    nc = bacc.Bacc(None, target_bir_lowering=False)
    x_in = nc.declare_dram_parameter("x", [BPC, F], BF16, isOutput=False)
    w_in = nc.declare_dram_parameter("w", [F, U], BF16, isOutput=False)
    out_ext = nc.declare_dram_parameter("out", [BPC, U], FP32, isOutput=True)

    with tile.TileContext(nc) as tc, ExitStack() as ctx:
        sb = ctx.enter_context(tc.tile_pool(name="sb", bufs=1))
        ps = ctx.enter_context(tc.tile_pool(name="ps", bufs=1, space="PSUM"))

        # ---- loads (two HWDGE rings: x on SP, w on ACT) ----
        xt = sb.tile([128, 2, F], BF16, tag="xt")       # xt[p, m, :] = x[m*128+p, :]
        nc.sync.dma_start(out=xt, in_=x_in.rearrange("(m p) f -> p m f", p=128))
        wt = sb.tile([128, KT, U], BF16, tag="wt")      # wt[p, k, :] = w[k*128+p, :]
        nc.scalar.dma_start(out=wt, in_=w_in.rearrange("(k p) u -> p k u", p=128))

        ident = sb.tile([128, 128], FP16, tag="ident")
        make_identity(nc, ident)
        alpha_col = sb.tile([128, 1], FP32, tag="alpha_col")
        nc.vector.memset(alpha_col, ALPHA)
        lnb_col = sb.tile([128, 1], FP32, tag="lnb_col")
        nc.vector.memset(lnb_col, 0.0)
        # row-select masks for the wred broadcast matmuls: e{r}[c, :] = (c == r)
        e0 = sb.tile([2, 128], FP16, tag="e0")
        e1 = sb.tile([2, 128], FP16, tag="e1")
        for r, e in ((0, e0), (1, e1)):
            nc.gpsimd.memset(e, 0.0)
            nc.gpsimd.affine_select(
                out=e, in_=e, compare_op=ALU.is_equal, fill=1.0,
                base=-r, pattern=[[0, 128]], channel_multiplier=1)

        # ---- PE warm-up: junk matmuls so HAM un-throttles before real work
        # (shares a PSUM bank with the later pTc/pTm scratch tiles — all are
        # PE-written and strictly sequential in PE program order)
        pwarm = ps.tile([128, 128], FP32, tag="scratch")
        for _ in range(NWARM):
            nc.tensor.matmul(out=pwarm, lhsT=ident, rhs=ident,
                             start=True, stop=True)

        # ---- x row stats ----
        mx = sb.tile([128, 2], FP32, tag="mx")
        mn = sb.tile([128, 2], FP32, tag="mn")
        nc.vector.tensor_reduce(out=mx, in_=xt, axis=X_AX, op=ALU.max)

        # ---- w chain: per-u normalizer wred = [T*max_f w | -T*min_f w] ----
        t01x = sb.tile([128, 2, 128], BF16, tag="t01x")
        t01n = sb.tile([128, 2, 128], BF16, tag="t01n")
        nc.vector.tensor_max(out=t01x, in0=wt[:, 0:2, 0:128], in1=wt[:, 2:4, 0:128])
        nc.vector.tensor_tensor(out=t01n, in0=wt[:, 0:2, 128:U],
                                in1=wt[:, 2:4, 128:U], op=ALU.min)
        comb = sb.tile([128, U], FP16, tag="comb")
        cx = sb.tile([128, 128], BF16, tag="cx")
        cn = sb.tile([128, 128], BF16, tag="cn")
        nc.vector.tensor_max(out=cx, in0=t01x[:, 0, :], in1=t01x[:, 1, :])
        nc.vector.tensor_tensor(out=cn, in0=t01n[:, 0, :], in1=t01n[:, 1, :],
                                op=ALU.min)
        nc.vector.tensor_scalar(out=comb[:, 0:128], in0=cx, scalar1=T,
                                scalar2=None, op0=ALU.mult)
        nc.vector.tensor_scalar(out=comb[:, 128:U], in0=cn, scalar1=-T,
                                scalar2=None, op0=ALU.mult)

        # cross-partition max of comb via PE: transpose -> reduce -> broadcast
        pTc = ps.tile([128, 2, 128], FP16, tag="scratch")
        nc.tensor.transpose(pTc[:, 0, :], comb[:, 0:128], ident)
        nc.tensor.transpose(pTc[:, 1, :], comb[:, 128:U], ident)
        mw = sb.tile([128, 2], FP32, tag="mw")
        nc.vector.tensor_reduce(out=mw, in_=pTc, axis=X_AX, op=ALU.max)
        mwh = sb.tile([128, 2], FP16, tag="mwh")
        nc.vector.tensor_scalar(out=mwh, in0=mw, scalar1=1.0, scalar2=None,
                                op0=ALU.mult)
        pTm = ps.tile([2, 128], FP16, tag="scratch")
        nc.tensor.transpose(pTm, mwh, ident)
        bdc = sb.tile([2, 128], FP16, tag="bdc")
        nc.vector.tensor_copy(out=bdc, in_=pTm)
        # broadcast row v of bdc to all 128 partitions of half v
        wredPS = ps.tile([128, U], FP32, tag="wredPS")
        nc.tensor.matmul(out=wredPS[:, 0:128], lhsT=e0, rhs=bdc,
                         start=True, stop=True)
        nc.tensor.matmul(out=wredPS[:, 128:U], lhsT=e1, rhs=bdc,
                         start=True, stop=True)

        # ---- x-side: xs = x - stat (fp16), transposed pre-exp ----
        nc.vector.tensor_reduce(out=mn, in_=xt, axis=X_AX, op=ALU.min)
        xsP = sb.tile([128, 2, F], FP16, tag="xsP")
        xsN = sb.tile([128, 2, F], FP16, tag="xsN")
        for m in range(2):
            nc.vector.tensor_scalar(out=xsP[:, m, :], in0=xt[:, m, :],
                                    scalar1=1.0, scalar2=mx[:, m:m + 1],
                                    op0=ALU.mult, op1=ALU.subtract)
            nc.vector.tensor_scalar(out=xsN[:, m, :], in0=xt[:, m, :],
                                    scalar1=1.0, scalar2=mn[:, m:m + 1],
                                    op0=ALU.mult, op1=ALU.subtract)

        # ---- w factors: ew = exp(+-T*w - wred + A), single dif tile ----
        dif = sb.tile([128, KT, U], FP16, tag="dif")
        wrbP = wredPS[:, 0:128].rearrange("p (o u) -> p o u", o=1) \
                               .to_broadcast((128, KT, 128))
        wrbN = wredPS[:, 128:U].rearrange("p (o u) -> p o u", o=1) \
                               .to_broadcast((128, KT, 128))
        nc.vector.scalar_tensor_tensor(out=dif[:, :, 0:128], in0=wt[:, :, 0:128],
                                       scalar=T, in1=wrbP,
                                       op0=ALU.mult, op1=ALU.subtract)
        nc.vector.scalar_tensor_tensor(out=dif[:, :, 128:U], in0=wt[:, :, 128:U],
                                       scalar=-T, in1=wrbN,
                                       op0=ALU.mult, op1=ALU.subtract)
        ew = sb.tile([128, KT, U], BF16, tag="ew")
        nc.scalar.activation(out=ew[:, :, 0:128], in_=dif[:, :, 0:128],
                             func=AF.Exp, bias=alpha_col, scale=1.0)
        nc.scalar.activation(out=ew[:, :, 128:U], in_=dif[:, :, 128:U],
                             func=AF.Exp, bias=alpha_col, scale=1.0)

        # ---- transpose xs, exp straight out of PSUM ----
        exT = {}
        for v, (xs, sgn) in enumerate(((xsP, T), (xsN, -T))):
            for m in range(2):
                pex = ps.tile([128, KT, 128], FP16, tag=f"pex{v}{m}")
                for k in range(KT):
                    nc.tensor.transpose(pex[:, k, :],
                                        xs[:, m, k * 128:(k + 1) * 128], ident)
                dst = sb.tile([128, KT, 128], BF16, tag=f"exT{v}{m}")
                nc.scalar.activation(out=dst, in_=pex, func=AF.Exp,
                                     bias=alpha_col, scale=sgn)
                exT[(v, m)] = dst

        # ---- matmuls: one accumulation group per (m, v) ----
        S = ps.tile([128, 4, 128], FP32, tag="S")       # slice 2m+v
        for m in range(2):
            for v in range(2):
                for k in range(KT):
                    nc.tensor.matmul(
                        out=S[:, 2 * m + v, :],
                        lhsT=exT[(v, m)][:, k, :],
                        rhs=ew[:, k, v * 128:(v + 1) * 128],
                        start=(k == 0), stop=(k == KT - 1))

        # ---- joint log-space epilogue on [128, 4, 128] ----
        bits = S.bitcast(I32)
        efi = sb.tile([128, 4, 128], I32, tag="efi")
        nc.vector.tensor_scalar(out=efi, in0=bits, scalar1=23, scalar2=None,
                                op0=ALU.arith_shift_right)
        ef = sb.tile([128, 4, 128], FP32, tag="ef")
        nc.vector.tensor_scalar(out=ef, in0=efi, scalar1=LN2, scalar2=None,
                                op0=ALU.mult)
        mant = sb.tile([128, 4, 128], FP32, tag="mant")
        nc.vector.tensor_scalar(out=mant.bitcast(I32), in0=bits,
                                scalar1=0x007FFFFF, scalar2=0x3F800000,
                                op0=ALU.bitwise_and, op1=ALU.bitwise_or)
        lnm = sb.tile([128, 4, 128], FP32, tag="lnm")
        nc.scalar.activation(out=lnm, in_=mant, func=AF.Ln,
                             bias=lnb_col, scale=1.0)
        # g1 = ef + wred ; t3 = lnm + g1   (wred broadcast over m)
        g1 = sb.tile([128, 2, U], FP32, tag="g1")
        wrb2 = wredPS.rearrange("p (o u) -> p o u", o=1).to_broadcast((128, 2, U))
        nc.vector.tensor_tensor(out=g1, in0=ef.rearrange("p (m v) u -> p m (v u)", v=2),
                                in1=wrb2, op=ALU.add)
        t3 = sb.tile([128, 2, U], FP32, tag="t3")
        nc.vector.scalar_tensor_tensor(
            out=t3, in0=lnm.rearrange("p (m v) u -> p m (v u)", v=2),
            scalar=0.0, in1=g1, op0=ALU.add, op1=ALU.add)
        # res = sgn*t3 + statadj ; ship per m
        CADJ = (2.0 * ALPHA + 127.0 * LN2) / T
        statP = sb.tile([128, 2], FP32, tag="statP")
        statN = sb.tile([128, 2], FP32, tag="statN")
        nc.vector.tensor_scalar(out=statP, in0=mx, scalar1=-CADJ, scalar2=None,
                                op0=ALU.add)
        nc.vector.tensor_scalar(out=statN, in0=mn, scalar1=CADJ, scalar2=None,
                                op0=ALU.add)
        res = sb.tile([128, 2, U], FP32, tag="res")
        ov = out_ext.rearrange("(m p) u -> p m u", p=128)
        for m in range(2):
            for v, (sgn, stat) in enumerate(((1.0 / T, statP), (-1.0 / T, statN))):
                nc.vector.tensor_scalar(
                    out=res[:, m, v * 128:(v + 1) * 128],
                    in0=t3[:, m, v * 128:(v + 1) * 128],
                    scalar1=sgn, scalar2=stat[:, m:m + 1],
                    op0=ALU.mult, op1=ALU.add)
            nc.sync.dma_start(out=ov[:, m, :], in_=res[:, m, :])

    nc.finalize()
    return nc


_NC = None


def _get_module() -> bass.Bass:
    global _NC
    if _NC is None:
        _NC = _build_module()
    return _NC


def kernel(x: np.ndarray, w: np.ndarray, _trace: bool = False, **_unused):
    assert x.shape == (2048, 512) and w.shape == (512, 256)
    xb = np.ascontiguousarray(x.astype(ml_dtypes.bfloat16))
    wb = np.ascontiguousarray(w.astype(ml_dtypes.bfloat16))
    nc = _get_module()
    in_maps = [
        {"x": xb[i * BPC:(i + 1) * BPC], "w": wb} for i in range(N_CORES)
    ]
    r = run_bass_kernel_spmd(nc, in_maps, list(range(N_CORES)), trace=_trace)
    out = np.concatenate([r.results[i]["out"] for i in range(N_CORES)], axis=0)
    if _trace:
        kernel.last_exec_time_ns = r.exec_time_ns
        kernel.last_results = r
    return out


# revision 8
# speedup vs baseline: 1.1210x; 1.0203x over previous
# Tropical (max/min-plus) pseudo-matmul kernel for Trainium2, SPMD over 8 cores.
#
#   out[b, u] = max_f(x[b,f] + w[f,u])   for u < 128
#   out[b, u] = min_f(x[b,f] + w[f,u])   for u >= 128
#
# Log-sum-exp on the PE array: with per-row/per-col normalizers mx[b], mw[u],
#   max_f(x+w) ~= mx + mw + (1/T)( ln( sum_f e^{T(x-mx)+A} e^{T(w-mw)+A} ) - 2A )
# i.e. a plain bf16 matmul of exponential factors accumulated in fp32.  The min
# half runs the same pipeline on negated data.  ln() is evaluated by splitting
# S = m * 2^e with integer ops so only the mantissa in [1,2) hits the Ln table.
#
# v2 layout choices (vs the earlier 32.8us version):
#  - inputs are cast to bf16 on the host: halves DMA bytes, doubles DVE rates.
#  - per-u normalizer via PE (transpose -> free-axis reduce -> ones-matmul
#    broadcast) instead of gpsimd partition_all_reduce (avoids ~6us IRAM load).
#  - x-side: subtract the row stat first (fp16), PE-transpose the *pre-exp*
#    values, then exp straight out of PSUM -> kills the PSUM->SBUF copies.
#  - single PSUM accumulation group over all 4 K-tiles (no block-max reduce).
#  - PE warm-up matmuls during the DMA window so real MMs run at 2.4 GHz.
# Batch is sharded 8 x 256 rows; w is replicated.
import numpy as np
import ml_dtypes
from contextlib import ExitStack

import concourse.bass as bass
import concourse.bacc as bacc
import concourse.tile as tile
from concourse import mybir
from concourse.bass_utils import run_bass_kernel_spmd
from concourse.masks import make_identity

FP32 = mybir.dt.float32
BF16 = mybir.dt.bfloat16
FP16 = mybir.dt.float16
I32 = mybir.dt.int32
AF = mybir.ActivationFunctionType
ALU = mybir.AluOpType
X_AX = mybir.AxisListType.X

T = 23.25       # LSE sharpness; limited by bf16 factor underflow on real data
ALPHA = 40.0    # per-factor exponent shift
LN2 = float(np.log(2.0))
N_CORES = 8
BPC = 256       # batch rows per core
F = 512
U = 256
KT = 4          # K tiles of 128
NWARM = 24      # PE warm-up matmuls (HAM un-throttle needs ~3.4us of activity)


def _patch_act_tables():
    """Make natural_log_exp_and_others the only table set providing Exp/Ln
    so the Bacc table-load pass emits a single ACT_TABLE_LOAD."""
    if getattr(bacc, "_act_tables_patched", False):
        return
    orig = bacc.get_activation_tables

    def patched(arch):
        t = dict(orig(arch))
        for name in list(t.keys()):
            if name != "natural_log_exp_and_others":
                t[name] = set(t[name]) - {AF.Exp, AF.Ln}
        return t

    bacc.get_activation_tables = patched
    bacc._act_tables_patched = True


def _build_module() -> bass.Bass:
    _patch_act_tables()
    nc = bacc.Bacc(None, target_bir_lowering=False)
    x_in = nc.declare_dram_parameter("x", [BPC, F], BF16, isOutput=False)
    w_in = nc.declare_dram_parameter("w", [F, U], BF16, isOutput=False)
    out_ext = nc.declare_dram_parameter("out", [BPC, U], FP32, isOutput=True)

    with tile.TileContext(nc) as tc, ExitStack() as ctx:
        sb = ctx.enter_context(tc.tile_pool(name="sb", bufs=1))
        ps = ctx.enter_context(tc.tile_pool(name="ps", bufs=1, space="PSUM"))

        # ---- loads (two HWDGE rings: x on SP, w on ACT) ----
        xt = sb.tile([128, 2, F], BF16, tag="xt")       # xt[p, m, :] = x[m*128+p, :]
        nc.sync.dma_start(out=xt, in_=x_in.rearrange("(m p) f -> p m f", p=128))
        wt = sb.tile([128, KT, U], BF16, tag="wt")      # wt[p, k, :] = w[k*128+p, :]
        nc.scalar.dma_start(out=wt, in_=w_in.rearrange("(k p) u -> p k u", p=128))

        ident = sb.tile([128, 128], FP16, tag="ident")
        make_identity(nc, ident)
        alpha_col = sb.tile([128, 1], FP32, tag="alpha_col")
        nc.vector.memset(alpha_col, ALPHA)
        lnb_col = sb.tile([128, 1], FP32, tag="lnb_col")
        nc.vector.memset(lnb_col, 0.0)
        # row-select masks for the wred broadcast matmuls: e{r}[c, :] = (c == r)
        e0 = sb.tile([2, 128], FP16, tag="e0")
        e1 = sb.tile([2, 128], FP16, tag="e1")
        for r, e in ((0, e0), (1, e1)):
            nc.gpsimd.memset(e, 0.0)
            # fill lands where the predicate is FALSE: row c == r gets 1.0
            nc.gpsimd.affine_select(
                out=e, in_=e, compare_op=ALU.not_equal, fill=1.0,
                base=-r, pattern=[[0, 128]], channel_multiplier=1)

        # ---- PE warm-up: junk matmuls so HAM un-throttles before real work
        # (shares a PSUM bank with the later pTc/pTm scratch tiles — all are
        # PE-written and strictly sequential in PE program order)
        pwarm = ps.tile([128, 128], FP32, tag="scratch")
        for _ in range(NWARM):
            nc.tensor.matmul(out=pwarm, lhsT=ident, rhs=ident,
                             start=True, stop=True)

        # ---- x row stats ----
        mx = sb.tile([128, 2], FP32, tag="mx")
        mn = sb.tile([128, 2], FP32, tag="mn")
        nc.vector.tensor_reduce(out=mx, in_=xt, axis=X_AX, op=ALU.max)

        # ---- w chain: per-u normalizer wred = [T*max_f w | -T*min_f w] ----
        t01x = sb.tile([128, 2, 128], BF16, tag="t01x")
        t01n = sb.tile([128, 2, 128], BF16, tag="t01n")
        nc.vector.tensor_max(out=t01x, in0=wt[:, 0:2, 0:128], in1=wt[:, 2:4, 0:128])
        nc.vector.tensor_tensor(out=t01n, in0=wt[:, 0:2, 128:U],
                                in1=wt[:, 2:4, 128:U], op=ALU.min)
        comb = sb.tile([128, U], FP16, tag="comb")
        cx = sb.tile([128, 128], BF16, tag="cx")
        cn = sb.tile([128, 128], BF16, tag="cn")
        nc.vector.tensor_max(out=cx, in0=t01x[:, 0, :], in1=t01x[:, 1, :])
        nc.vector.tensor_tensor(out=cn, in0=t01n[:, 0, :], in1=t01n[:, 1, :],
                                op=ALU.min)
        nc.vector.tensor_scalar(out=comb[:, 0:128], in0=cx, scalar1=T,
                                scalar2=None, op0=ALU.mult)
        nc.vector.tensor_scalar(out=comb[:, 128:U], in0=cn, scalar1=-T,
                                scalar2=None, op0=ALU.mult)

        # cross-partition max of comb via PE: transpose -> reduce -> broadcast
        pTc = ps.tile([128, 2, 128], FP16, tag="scratch")
        nc.tensor.transpose(pTc[:, 0, :], comb[:, 0:128], ident)
        nc.tensor.transpose(pTc[:, 1, :], comb[:, 128:U], ident)
        mw = sb.tile([128, 2], FP32, tag="mw")
        nc.vector.tensor_reduce(out=mw, in_=pTc, axis=X_AX, op=ALU.max)
        mwh = sb.tile([128, 2], FP16, tag="mwh")
        nc.vector.tensor_scalar(out=mwh, in0=mw, scalar1=1.0, scalar2=None,
                                op0=ALU.mult)
        pTm = ps.tile([2, 128], FP16, tag="scratch")
        nc.tensor.transpose(pTm, mwh, ident)
        bdc = sb.tile([2, 128], FP16, tag="bdc")
        nc.vector.tensor_copy(out=bdc, in_=pTm)
        # broadcast row v of bdc to all 128 partitions of half v
        wredPS = ps.tile([128, U], FP32, tag="wredPS")
        nc.tensor.matmul(out=wredPS[:, 0:128], lhsT=e0, rhs=bdc,
                         start=True, stop=True)
        nc.tensor.matmul(out=wredPS[:, 128:U], lhsT=e1, rhs=bdc,
                         start=True, stop=True)

        # ---- x-side: xs = x - stat (fp16), transposed pre-exp ----
        nc.vector.tensor_reduce(out=mn, in_=xt, axis=X_AX, op=ALU.min)
        xsP = sb.tile([128, 2, F], FP16, tag="xsP")
        xsN = sb.tile([128, 2, F], FP16, tag="xsN")
        for m in range(2):
            nc.vector.tensor_scalar(out=xsP[:, m, :], in0=xt[:, m, :],
                                    scalar1=1.0, scalar2=mx[:, m:m + 1],
                                    op0=ALU.mult, op1=ALU.subtract)
            nc.vector.tensor_scalar(out=xsN[:, m, :], in0=xt[:, m, :],
                                    scalar1=1.0, scalar2=mn[:, m:m + 1],
                                    op0=ALU.mult, op1=ALU.subtract)

        # ---- w factors: ew = exp(+-T*w - wred + A), single dif tile ----
        dif = sb.tile([128, KT, U], FP16, tag="dif")
        wrbP = wredPS[:, 0:128].rearrange("p (o u) -> p o u", o=1) \
                               .to_broadcast((128, KT, 128))
        wrbN = wredPS[:, 128:U].rearrange("p (o u) -> p o u", o=1) \
                               .to_broadcast((128, KT, 128))
        nc.vector.scalar_tensor_tensor(out=dif[:, :, 0:128], in0=wt[:, :, 0:128],
                                       scalar=T, in1=wrbP,
                                       op0=ALU.mult, op1=ALU.subtract)
        nc.vector.scalar_tensor_tensor(out=dif[:, :, 128:U], in0=wt[:, :, 128:U],
                                       scalar=-T, in1=wrbN,
                                       op0=ALU.mult, op1=ALU.subtract)
        ew = sb.tile([128, KT, U], BF16, tag="ew")
        nc.scalar.activation(out=ew[:, :, 0:128], in_=dif[:, :, 0:128],
                             func=AF.Exp, bias=alpha_col, scale=1.0)
        nc.scalar.activation(out=ew[:, :, 128:U], in_=dif[:, :, 128:U],
                             func=AF.Exp, bias=alpha_col, scale=1.0)

        # ---- transpose xs, exp straight out of PSUM ----
        exT = {}
        for v, (xs, sgn) in enumerate(((xsP, T), (xsN, -T))):
            for m in range(2):
                pex = ps.tile([128, KT, 128], FP16, tag=f"pex{v}{m}")
                for k in range(KT):
                    nc.tensor.transpose(pex[:, k, :],
                                        xs[:, m, k * 128:(k + 1) * 128], ident)
                dst = sb.tile([128, KT, 128], BF16, tag=f"exT{v}{m}")
                nc.scalar.activation(out=dst, in_=pex, func=AF.Exp,
                                     bias=alpha_col, scale=sgn)
                exT[(v, m)] = dst

        # ---- matmuls: one accumulation group per (m, v) ----
        S = ps.tile([128, 4, 128], FP32, tag="S")       # slice 2m+v
        for m in range(2):
            for v in range(2):
                for k in range(KT):
                    nc.tensor.matmul(
                        out=S[:, 2 * m + v, :],
                        lhsT=exT[(v, m)][:, k, :],
                        rhs=ew[:, k, v * 128:(v + 1) * 128],
                        start=(k == 0), stop=(k == KT - 1))

        # ---- joint log-space epilogue on [128, 4, 128] ----
        bits = S.bitcast(I32)
        efi = sb.tile([128, 4, 128], I32, tag="efi")
        nc.vector.tensor_scalar(out=efi, in0=bits, scalar1=23, scalar2=None,
                                op0=ALU.arith_shift_right)
        ef = sb.tile([128, 4, 128], FP32, tag="ef")
        nc.vector.tensor_scalar(out=ef, in0=efi, scalar1=LN2, scalar2=None,
                                op0=ALU.mult)
        mant = sb.tile([128, 4, 128], FP32, tag="mant")
        nc.vector.tensor_scalar(out=mant.bitcast(I32), in0=bits,
                                scalar1=0x007FFFFF, scalar2=0x3F800000,
                                op0=ALU.bitwise_and, op1=ALU.bitwise_or)
        lnm = sb.tile([128, 4, 128], FP32, tag="lnm")
        nc.scalar.activation(out=lnm, in_=mant, func=AF.Ln,
                             bias=lnb_col, scale=1.0)
        # g1 = ef + wred ; t3 = lnm + g1   (wred broadcast over m)
        g1 = sb.tile([128, 2, U], FP32, tag="g1")
        wrb2 = wredPS.rearrange("p (o u) -> p o u", o=1).to_broadcast((128, 2, U))
        nc.vector.tensor_tensor(out=g1, in0=ef.rearrange("p (m v) u -> p m (v u)", v=2),
                                in1=wrb2, op=ALU.add)
        t3 = sb.tile([128, 2, U], FP32, tag="t3")
        nc.vector.scalar_tensor_tensor(
            out=t3, in0=lnm.rearrange("p (m v) u -> p m (v u)", v=2),
            scalar=0.0, in1=g1, op0=ALU.add, op1=ALU.add)
        # res = sgn*t3 + statadj ; ship per m
        CADJ = (2.0 * ALPHA + 127.0 * LN2) / T
        statP = sb.tile([128, 2], FP32, tag="statP")
        statN = sb.tile([128, 2], FP32, tag="statN")
        nc.vector.tensor_scalar(out=statP, in0=mx, scalar1=-CADJ, scalar2=None,
                                op0=ALU.add)
        nc.vector.tensor_scalar(out=statN, in0=mn, scalar1=CADJ, scalar2=None,
                                op0=ALU.add)
        res = sb.tile([128, 2, U], FP32, tag="res")
        ov = out_ext.rearrange("(m p) u -> p m u", p=128)
        for m in range(2):
            for v, (sgn, stat) in enumerate(((1.0 / T, statP), (-1.0 / T, statN))):
                nc.vector.tensor_scalar(
                    out=res[:, m, v * 128:(v + 1) * 128],
                    in0=t3[:, m, v * 128:(v + 1) * 128],
                    scalar1=sgn, scalar2=stat[:, m:m + 1],
                    op0=ALU.mult, op1=ALU.add)
            nc.sync.dma_start(out=ov[:, m, :], in_=res[:, m, :])

    nc.finalize()
    return nc


_NC = None


def _get_module() -> bass.Bass:
    global _NC
    if _NC is None:
        _NC = _build_module()
    return _NC


def kernel(x: np.ndarray, w: np.ndarray, _trace: bool = False, **_unused):
    assert x.shape == (2048, 512) and w.shape == (512, 256)
    xb = np.ascontiguousarray(x.astype(ml_dtypes.bfloat16))
    wb = np.ascontiguousarray(w.astype(ml_dtypes.bfloat16))
    nc = _get_module()
    in_maps = [
        {"x": xb[i * BPC:(i + 1) * BPC], "w": wb} for i in range(N_CORES)
    ]
    r = run_bass_kernel_spmd(nc, in_maps, list(range(N_CORES)), trace=_trace)
    out = np.concatenate([r.results[i]["out"] for i in range(N_CORES)], axis=0)
    if _trace:
        kernel.last_exec_time_ns = r.exec_time_ns
        kernel.last_results = r
    return out
